# revision 34
# baseline (speedup 1.0000x reference)
"""Trainium2 Bass kernel for nn_DecoderLayer (Mamba block + BitNet FFN).

v2: channel-parallel mamba (256 ch/core) in bf16 -> bf16 AllReduce (xproj
rows) -> DVE tensor_tensor_scan over (d,n) lanes with dt-chunked AllToAll
(d-shard -> t-shard) overlapping the second half of the scan -> sequence-
parallel out_proj + rmsnorm + BitNet FFN (host-prequantized ternary weights,
exact integer bf16 matmuls) with weights prefetched under the scan window.
"""
import numpy as np
import ml_dtypes

try:
    import jax
    jax.config.update("jax_compilation_cache_dir", "/root/jaxcache")
    jax.config.update("jax_persistent_cache_min_compile_time_secs", 1.0)
except Exception:
    pass

import concourse.bass as bass
import concourse.mybir as mybir
import concourse.tile as tile
from concourse import bacc
import concourse.bass_utils as _BU
from concourse.bass_utils import run_bass_kernel_spmd


BF16 = mybir.dt.bfloat16
F32 = mybir.dt.float32
AF = mybir.ActivationFunctionType
OP = mybir.AluOpType

L, DM, DI, DS, DC, DTR, DFF = 2048, 1024, 2048, 16, 4, 64, 4096
EPS = 1e-6
NCORES = 8
DIC = DI // NCORES   # 256 channels per core
NDT = DIC // 128     # 2 d-tiles
LT = L // NCORES     # 256 tokens per core
NTT = LT // 128      # 2 token-tiles
MAGIC = 12582912.0   # 1.5*2^23: x+M-M == rint(x) for |x|<2^22

_NC_CACHE = {}


def _bcast_row(src):
    """AP reading one row replicated across 128 partitions (stride-0)."""
    return bass.AP(tensor=src.tensor, offset=src.offset,
                   ap=[[0, 128]] + [list(p) for p in src.ap[1:]])


def _emit(nc, tc, ctx, g1, g2, dbg=False):
    import contextlib
    RG = [list(range(NCORES))]

    xT = nc.dram_tensor("xT", [DM, L], BF16, kind="ExternalInput")
    x_tok = nc.dram_tensor("x_tok", [LT, DM], F32, kind="ExternalInput")
    winT = nc.dram_tensor("winT", [DM, 2 * 128 * NDT], BF16, kind="ExternalInput")
    convw = nc.dram_tensor("convw", [DIC, DC], F32, kind="ExternalInput")
    convb = nc.dram_tensor("convb", [DIC, 1], F32, kind="ExternalInput")
    wxpT = nc.dram_tensor("wxpT", [DIC, 96], BF16, kind="ExternalInput")
    wdtT = nc.dram_tensor("wdtT", [DTR, DIC], BF16, kind="ExternalInput")
    bdt = nc.dram_tensor("bdt", [DIC, 1], F32, kind="ExternalInput")
    acol = nc.dram_tensor("acol", [DIC, DS], F32, kind="ExternalInput")
    dpv = nc.dram_tensor("dpv", [DIC, 1], F32, kind="ExternalInput")
    woutT = nc.dram_tensor("woutT", [DI, DM], BF16, kind="ExternalInput")
    n1w = nc.dram_tensor("n1w", [1, DM], F32, kind="ExternalInput")
    n2w = nc.dram_tensor("n2w", [1, DM], F32, kind="ExternalInput")
    w1qT = nc.dram_tensor("w1qT", [DM, DFF], BF16, kind="ExternalInput")
    w2qT = nc.dram_tensor("w2qT", [DFF, DM], BF16, kind="ExternalInput")
    out_t = nc.dram_tensor("out", [LT, DM], F32, kind="ExternalOutput")
    dbg_t = {}
    if dbg:
        for nm, shp, dty in [("dbg_u", [128, L], BF16), ("dbg_dbl", [96, L], BF16),
                             ("dbg_delta", [128, L], BF16), ("dbg_yh", [128, L], BF16),
                             ("dbg_hps", [128, DM], F32), ("dbg_x1", [128, DM], F32),
                             ("dbg_f", [128, DFF], BF16)]:
            dbg_t[nm] = nc.dram_tensor(nm, shp, dty, kind="ExternalOutput")

    singles = ctx.enter_context(tc.tile_pool(name="singles", bufs=1))
    dram = ctx.enter_context(tc.tile_pool(name="dram", bufs=1, space="DRAM"))
    w1pool = ctx.enter_context(tc.tile_pool(name="w1p", bufs=1))
    oproj_stack = contextlib.ExitStack()
    opool = oproj_stack.enter_context(tc.tile_pool(name="oproj", bufs=1))
    psA_stack = contextlib.ExitStack()
    psum_small = psA_stack.enter_context(
        tc.tile_pool(name="psA", bufs=2, space="PSUM"))
    act_stack = contextlib.ExitStack()
    actpool = act_stack.enter_context(tc.tile_pool(name="acts", bufs=1))

    # ---- small per-partition constants
    convw_sb, convb_sb, bdt_sb, acol_sb, dp_sb = [], [], [], [], []
    for dt in range(NDT):
        sl = slice(dt * 128, (dt + 1) * 128)
        t1 = singles.tile([128, DC], F32, name=f"cw{dt}")
        nc.sync.dma_start(t1[:, :], convw[sl, :])
        convw_sb.append(t1)
        t2 = singles.tile([128, 1], F32, name=f"cb{dt}")
        nc.sync.dma_start(t2[:, :], convb[sl, :])
        convb_sb.append(t2)
        t3 = singles.tile([128, 1], F32, name=f"bd{dt}")
        nc.sync.dma_start(t3[:, :], bdt[sl, :])
        bdt_sb.append(t3)
        t4 = singles.tile([128, DS], F32, name=f"ac{dt}")
        nc.sync.dma_start(t4[:, :], acol[sl, :])
        acol_sb.append(t4)
        t5 = singles.tile([128, 1], F32, name=f"dp{dt}")
        nc.sync.dma_start(t5[:, :], dpv[sl, :])
        dp_sb.append(t5)
    wxpT_sb = singles.tile([128, NDT, 96], BF16)
    nc.sync.dma_start(wxpT_sb[:, :, :],
                      wxpT.rearrange("(k p) m -> p k m", p=128))
    wdtT_sb = singles.tile([DTR, DIC], BF16)
    nc.sync.dma_start(wdtT_sb[:, :], wdtT[:, :])
    ident_bf = singles.tile([128, 128], BF16)
    from concourse.masks import make_identity
    make_identity(nc, ident_bf[:, :])

    # ================= PHASE A: in_proj (channel-parallel) =================
    conv_stack = contextlib.ExitStack()
    convpool = conv_stack.enter_context(tc.tile_pool(name="convp", bufs=1))
    init_stack = contextlib.ExitStack()
    init_pool = init_stack.enter_context(tc.tile_pool(name="init", bufs=1))
    # winT (1MB) first so it lands before the 4MB xT halves; xT split in two
    # k-half tiles so in_proj's first k-group starts at the half-way mark
    winT_sb = init_pool.tile([128, 8, 2 * 128 * NDT], BF16)
    nc.sync.dma_start(winT_sb[:, :, :],
                      winT.rearrange("(k p) m -> p k m", p=128))
    xTr = xT.rearrange("(k p) l -> p k l", p=128)
    xT_h = []
    for h in range(2):
        xh = init_pool.tile([128, 4, L], BF16, name=f"xTh{h}")
        nc.sync.dma_start(xh[:, :, :], xTr[:, h * 4:(h + 1) * 4, :])
        xT_h.append(xh)

    u_pad, sz = [], []
    for dt in range(NDT):
        up = convpool.tile([128, L + 3], BF16, name=f"upad{dt}")
        nc.vector.memset(up[:, 0:3], 0.0)
        u_pad.append(up)
        sz.append(actpool.tile([128, L], BF16, name=f"sz{dt}"))

    def _inproj_mtile(mt):
        # m-tiles: 0..NDT-1 u, NDT..2*NDT-1 z
        for c in range(L // 512):
            ps = psum_small.tile([128, 512], F32, tag="psA")
            for k in range(8):
                nc.tensor.matmul(
                    ps[:, :],
                    winT_sb[:, k, mt * 128:(mt + 1) * 128],
                    xT_h[k // 4][:, k % 4, c * 512:(c + 1) * 512],
                    start=(k == 0), stop=(k == 7))
            if mt < NDT:
                nc.scalar.copy(
                    u_pad[mt][:, 3 + c * 512: 3 + (c + 1) * 512], ps[:, :])
            else:
                nc.scalar.activation(
                    sz[mt - NDT][:, c * 512:(c + 1) * 512], ps[:, :], AF.Silu)

    def _conv(dt):
        ca = convpool.tile([128, L], BF16, name=f"cva{dt}", tag="cva")
        cb = convpool.tile([128, L], BF16, name=f"cvb{dt}", tag="cvb")
        nc.vector.tensor_scalar_mul(ca[:, :], u_pad[dt][:, 0:L],
                                    convw_sb[dt][:, 0:1])
        nc.vector.scalar_tensor_tensor(
            cb[:, :], u_pad[dt][:, 1:L + 1], convw_sb[dt][:, 1:2], ca[:, :],
            op0=OP.mult, op1=OP.add)
        nc.vector.scalar_tensor_tensor(
            ca[:, :], u_pad[dt][:, 2:L + 2], convw_sb[dt][:, 2:3], cb[:, :],
            op0=OP.mult, op1=OP.add)
        nc.vector.scalar_tensor_tensor(
            cb[:, :], u_pad[dt][:, 3:L + 3], convw_sb[dt][:, 3:4], ca[:, :],
            op0=OP.mult, op1=OP.add)
        ua = convpool.tile([128, L], BF16, name=f"uact{dt}")
        nc.scalar.activation(ua[:, :], cb[:, :], AF.Silu,
                             bias=convb_sb[dt][:, 0:1])
        u_act.append(ua)

    # u-tiles + conv feed the AllReduce ASAP; z0 fills PE during conv1;
    # z1 runs under the AllReduce
    u_act = []
    _inproj_mtile(0)
    _conv(0)
    _inproj_mtile(1)
    _inproj_mtile(NDT)       # z0
    _conv(1)

    # ---- xproj partial (bf16) + bf16 AllReduce ----
    dbl_loc = convpool.tile([96, L], BF16, name="dbl_loc")
    for c in range(L // 512):
        ps = psum_small.tile([96, 512], F32, tag="psA0")
        for kt in range(NDT):
            nc.tensor.matmul(
                ps[:, :],
                wxpT_sb[:, kt, :],
                u_act[kt][:, c * 512:(c + 1) * 512],
                start=(kt == 0), stop=(kt == NDT - 1))
        nc.scalar.copy(dbl_loc[:, c * 512:(c + 1) * 512], ps[:, :])

    if dbg:
        nc.sync.dma_start(dbg_t["dbg_u"][:, :], u_act[0][:, :])
    ar_i = dram.tile([96, L], BF16)
    ar_o = dram.tile([96, L], BF16, addr_space="Shared")
    nc.sync.dma_start(ar_i[:, :], dbl_loc[:, :])
    nc.gpsimd.collective_compute("AllReduce", OP.add, replica_groups=RG,
                                 ins=[ar_i.opt()], outs=[ar_o.opt()])
    # z1 in_proj + gate constant Dp*u*silu(z) overlap the AllReduce
    _inproj_mtile(NDT + 1)
    init_stack.close()
    gu_bf = []
    for dt in range(NDT):
        gt = convpool.tile([128, L], BF16, name=f"gut{dt}")
        nc.scalar.activation(gt[:, :], u_act[dt][:, :], AF.Copy,
                             scale=dp_sb[dt][:, 0:1])
        gu = actpool.tile([128, L], BF16, name=f"gu{dt}")
        nc.vector.tensor_tensor(gu[:, :], gt[:, :], sz[dt][:, :], op=OP.mult)
        gu_bf.append(gu)

    dbl_sb = convpool.tile([96, L], BF16, name="dbl_sb")
    nc.sync.dma_start(dbl_sb[:, :], ar_o[:, :])

    if dbg:
        nc.sync.dma_start(dbg_t["dbg_dbl"][:, :], dbl_sb[:, :])
    # B/C rows -> DRAM bounce for partition-replication (already bf16)
    bcb = dram.tile([32, L], BF16)
    nc.sync.dma_start(bcb[:, :], dbl_sb[64:96, :])
    # prefetch BitNet w1 under the scan window (after the AllReduce so the
    # 8MB burst doesn't contend with the collective's ring DMA)
    w1qT_sb = w1pool.tile([128, 8, DFF], BF16)
    nc.sync.dma_start(w1qT_sb[:, :, :], w1qT.rearrange("(k p) j -> p k j", p=128))

    # ================= delta = softplus(wdt @ dt + bdt) =================
    delta = []
    for dt in range(NDT):
        dl = actpool.tile([128, L], BF16, name=f"delta{dt}")
        for c in range(L // 512):
            ps = psum_small.tile([128, 512], F32, tag="psA0")
            nc.tensor.matmul(
                ps[:, :],
                wdtT_sb[:, dt * 128:(dt + 1) * 128],
                dbl_sb[0:DTR, c * 512:(c + 1) * 512],
                start=True, stop=True)
            # exp(x + bdt) from PSUM, then ln(1+e) in-place
            nc.scalar.activation(dl[:, c * 512:(c + 1) * 512], ps[:, :],
                                 AF.Exp, bias=bdt_sb[dt][:, 0:1])
        nc.scalar.activation(dl[:, :], dl[:, :], AF.Ln, bias=1.0)
        delta.append(dl)

    if dbg:
        nc.sync.dma_start(dbg_t["dbg_delta"][:, :], delta[0][:, :])
    # delta*u in bf16
    du_bf = []
    for dt in range(NDT):
        db = actpool.tile([128, L], BF16, name=f"dubf{dt}")
        nc.vector.tensor_tensor(db[:, :], delta[dt][:, :], u_act[dt][:, :],
                                op=OP.mult)
        du_bf.append(db)
    conv_stack.close()

    # ================= scan over n (16 states), dt-major =================
    psA_stack.close()
    opb_stack = contextlib.ExitStack()
    opb_pool = opb_stack.enter_context(
        tc.tile_pool(name="opb", bufs=2, space="PSUM"))
    yps_stack = contextlib.ExitStack()
    y_ps_pool = yps_stack.enter_context(
        tc.tile_pool(name="yps", bufs=1, space="PSUM"))

    x_tok_sb = singles.tile([128, NTT, DM], F32)
    nc.sync.dma_start(x_tok_sb[:, :, :],
                      x_tok.rearrange("(tt p) m -> p tt m", p=128))
    n1w_rep = singles.tile([128, DM], F32)
    nc.sync.dma_start(n1w_rep[:, :], _bcast_row(n1w[0:1, :]))
    n2w_rep = singles.tile([128, DM], F32)
    nc.sync.dma_start(n2w_rep[:, :], _bcast_row(n2w[0:1, :]))

    scan_stack = contextlib.ExitStack()
    scanp = scan_stack.enter_context(tc.tile_pool(name="scanp", bufs=2))
    repp = scan_stack.enter_context(tc.tile_pool(name="repp", bufs=2))
    gatep = scan_stack.enter_context(tc.tile_pool(name="gatep", bufs=1))
    wsp = scan_stack.enter_context(tc.tile_pool(name="wstream", bufs=4))
    woutr = woutT.rearrange("(k p) m -> p k m", p=128)
    hps_l = [opb_pool.tile([128, DM], F32, name=f"hps{tt}", tag="hps")
             for tt in range(NTT)]

    a2a_i = [dram.tile([NCORES * 128, LT], BF16, name=f"a2ai{d}") for d in range(NDT)]
    a2a_o = [dram.tile([NCORES * 128, LT], BF16, name=f"a2ao{d}") for d in range(NDT)]
    yfull = []

    for dt in range(NDT):
        y_ps = y_ps_pool.tile([128, L], F32, name=f"yps{dt}", tag="yps")
        for n in range(DS):
            brep = repp.tile([128, L], BF16, name=f"brep{dt}_{n}", tag="brep")
            nc.sync.dma_start(brep[:, :], _bcast_row(bcb[n:n + 1, :]))
            crep = repp.tile([128, L], BF16, name=f"crep{dt}_{n}", tag="crep")
            nc.sync.dma_start(crep[:, :], _bcast_row(bcb[16 + n:17 + n, :]))
            dA = scanp.tile([128, L], BF16, name=f"dA{dt}_{n}", tag="dA")
            nc.scalar.activation(dA[:, :], delta[dt][:, :], AF.Exp,
                                 scale=acol_sb[dt][:, n:n + 1])
            dBu = scanp.tile([128, L], BF16, name=f"dBu{dt}_{n}", tag="dBu")
            nc.vector.tensor_tensor(dBu[:, :], du_bf[dt][:, :], brep[:, :],
                                    op=OP.mult)
            h = scanp.tile([128, L], BF16, name=f"h{dt}_{n}", tag="h")
            nc.vector.tensor_tensor_scan(h[:, :], dA[:, :], dBu[:, :], 0.0,
                                         OP.mult, OP.add)
            yt = scanp.tile([128, L], BF16, name=f"yt{dt}_{n}", tag="yt")
            nc.vector.tensor_tensor(yt[:, :], h[:, :], crep[:, :], op=OP.mult)
            for c in range(L // 512):
                nc.tensor.matmul(
                    y_ps[:, c * 512:(c + 1) * 512],
                    ident_bf[:, :],
                    yt[:, c * 512:(c + 1) * 512],
                    start=(n == 0), stop=(n == DS - 1),
                    skip_group_check=True)

        # gate: yhat = y*sz + Dp*u*sz, then dt-half AllToAll
        ygs = gatep.tile([128, L], BF16, name=f"ygs{dt}", tag="ygs")
        nc.vector.tensor_tensor(ygs[:, :], y_ps[:, :], sz[dt][:, :], op=OP.mult)
        yh = gatep.tile([128, L], BF16, name=f"yhat{dt}", tag="yhat")
        nc.vector.tensor_tensor(yh[:, :], ygs[:, :], gu_bf[dt][:, :], op=OP.add)
        if dbg and dt == 0:
            nc.sync.dma_start(dbg_t["dbg_yh"][:, :], yh[:, :])
        nc.sync.dma_start(
            a2a_i[dt].rearrange("(j c) t -> c j t", c=128)[:, :, :],
            yh.rearrange("c (j t) -> c j t", j=NCORES))
        nc.gpsimd.collective_compute("AllToAll", OP.bypass, replica_groups=RG,
                                     ins=[a2a_i[dt].opt()], outs=[a2a_o[dt].opt()])
        yf = opool.tile([128, NCORES, LT], BF16, name=f"yfull{dt}")
        nc.sync.dma_start(yf[:, :, :], a2a_o[dt].rearrange("(j p) t -> p j t", p=128))
        yfull.append(yf)

        # out_proj k-accumulation for this dt-half (overlaps remaining scan)
        for j in range(NCORES):
            wg = wsp.tile([128, DM], BF16, name=f"wg{dt}_{j}", tag="wg")
            nc.sync.dma_start(wg[:, :], woutr[:, 2 * j + dt, :])
            for tt in range(NTT):
                for mc in range(DM // 512):
                    nc.tensor.matmul(
                        hps_l[tt][:, mc * 512:(mc + 1) * 512],
                        yf[:, j, tt * 128:(tt + 1) * 128],
                        wg[:, mc * 512:(mc + 1) * 512],
                        start=(dt == 0 and j == 0),
                        stop=(dt == NDT - 1 and j == NCORES - 1),
                        skip_group_check=True)


    # scan pools done: free SBUF for phase B
    scan_stack.close()
    act_stack.close()
    yps_stack.close()

    oproj_stack.close()
    bpool = ctx.enter_context(tc.tile_pool(name="bpool", bufs=1))

    # ================= PHASE B (sequence-parallel, my LT tokens) ==========
    x1_l, scl1_l, xqT_l, fqT_l, scl2_l = [], [], [], [], []

    for tt in range(NTT):
        s = bpool.tile([128, DM], F32, name=f"s{tt}", tag="s")
        nc.vector.tensor_tensor(s[:, :], x_tok_sb[:, tt, :], hps_l[tt][:, :],
                                op=OP.add)
        sq = bpool.tile([128, DM], BF16, name=f"sq{tt}", tag="sqd")
        ssum = bpool.tile([128, 1], F32, name=f"ssum{tt}", tag="ssum")
        nc.scalar.activation(sq[:, :], s[:, :], AF.Square, accum_out=ssum[:, 0:1])
        v = bpool.tile([128, 1], F32, name=f"v{tt}", tag=f"v{tt}")
        nc.vector.tensor_scalar(v[:, :], ssum[:, :], 1.0 / DM, EPS,
                                op0=OP.mult, op1=OP.add)
        nc.scalar.activation(v[:, :], v[:, :], AF.Ln)
        nc.scalar.activation(v[:, :], v[:, :], AF.Exp, scale=-0.5)
        x1 = bpool.tile([128, DM], BF16, name=f"x1_{tt}", tag=f"x1_{tt}")
        xqT = bpool.tile([128, DM // 128, 128], BF16, name=f"xqT{tt}",
                         tag=f"xqT{tt}")
        for h in range(2):
            sl = slice(h * 512, (h + 1) * 512)
            nc.vector.scalar_tensor_tensor(x1[:, sl], s[:, sl], v[:, 0:1],
                                           n1w_rep[:, sl],
                                           op0=OP.mult, op1=OP.mult)
            nc.sync.dma_start_transpose(xqT[:, h * 4:(h + 1) * 4, :], x1[:, sl])
        x1_l.append(x1)
        if dbg and tt == 0:
            nc.sync.dma_start(dbg_t["dbg_hps"][:, :], s[:, :])
        xqT_l.append(xqT)

    opb_stack.close()
    w2pool = ctx.enter_context(tc.tile_pool(name="w2p", bufs=2))
    w2r = w2qT.rearrange("(k p) m -> p k m", p=128)
    KG = 4
    KPG = (DFF // 128) // KG
    psB_stack = contextlib.ExitStack()
    psB = psB_stack.enter_context(tc.tile_pool(name="psB", bufs=1, space="PSUM"))
    # ---- FFN mm1 (k-outer: stationary xqT reused) + gelu + quant2 ----
    for tt in range(NTT):
        f_sb = bpool.tile([128, DFF], BF16, name=f"f{tt}", tag="f")
        fps_l = [psB.tile([128, 512], F32, name=f"fps{tt}_{jc}", tag=f"fps{jc}")
                 for jc in range(DFF // 512)]
        for k in range(DM // 128):
            for jc in range(DFF // 512):
                nc.tensor.matmul(
                    fps_l[jc][:, :], xqT_l[tt][:, k, :],
                    w1qT_sb[:, k, jc * 512:(jc + 1) * 512],
                    start=(k == 0), stop=(k == DM // 128 - 1))
        for jc in range(DFF // 512):
            nc.scalar.activation(f_sb[:, jc * 512:(jc + 1) * 512],
                                 fps_l[jc][:, :],
                                 AF.Gelu_apprx_tanh, scale=g1)
        if dbg and tt == 0:
            nc.sync.dma_start(dbg_t["dbg_f"][:, :], f_sb[:, :])
        fqT = bpool.tile([128, DFF // 128, 128], BF16, name=f"fqT{tt}",
                         tag=f"fqT{tt}")
        for kg in range(4):
            nc.sync.dma_start_transpose(
                fqT[:, kg * 8:(kg + 1) * 8, :],
                f_sb[:, kg * 1024:(kg + 1) * 1024])
        fqT_l.append(fqT)

    # ---- FFN mm2 (w2 streamed in k-groups) + residual + rmsnorm ----
    psB_stack.close()
    psB2 = ctx.enter_context(tc.tile_pool(name="psB2", bufs=1, space="PSUM"))
    ops_t = [psB2.tile([128, DM], F32, name=f"ops{tt}") for tt in range(NTT)]
    for kg in range(KG):
        w2g = w2pool.tile([128, KPG, DM], BF16, name=f"w2g{kg}", tag="w2s")
        nc.sync.dma_start(w2g[:, :, :], w2r[:, kg * KPG:(kg + 1) * KPG, :])
        for tt in range(NTT):
            for kk in range(KPG):
                for mc in range(DM // 512):
                    nc.tensor.matmul(
                        ops_t[tt][:, mc * 512:(mc + 1) * 512],
                        fqT_l[tt][:, kg * KPG + kk, :],
                        w2g[:, kk, mc * 512:(mc + 1) * 512],
                        start=(kg == 0 and kk == 0),
                        stop=(kg == KG - 1 and kk == KPG - 1),
                        skip_group_check=True)
    for tt in range(NTT):
        o2 = bpool.tile([128, DM], F32, name=f"o2{tt}", tag="o2")
        nc.vector.scalar_tensor_tensor(
            o2[:, :], ops_t[tt][:, :], g2,
            x1_l[tt][:, :], op0=OP.mult, op1=OP.add)
        sq2 = bpool.tile([128, DM], BF16, name=f"sq2{tt}", tag="sqd")
        ssum2 = bpool.tile([128, 1], F32, name=f"ssum2{tt}", tag="ssum2")
        nc.scalar.activation(sq2[:, :], o2[:, :], AF.Square,
                             accum_out=ssum2[:, 0:1])
        v2 = bpool.tile([128, 1], F32, name=f"v2{tt}", tag=f"v2{tt}")
        nc.vector.tensor_scalar(v2[:, :], ssum2[:, :], 1.0 / DM, EPS,
                                op0=OP.mult, op1=OP.add)
        nc.scalar.activation(v2[:, :], v2[:, :], AF.Ln)
        nc.scalar.activation(v2[:, :], v2[:, :], AF.Exp, scale=-0.5)
        o2s = bpool.tile([128, DM], F32, name=f"o2s{tt}", tag="o2s")
        nc.scalar.activation(o2s[:, :], o2[:, :], AF.Copy, scale=v2[:, 0:1])
        ot = bpool.tile([128, DM], F32, name=f"ot{tt}", tag="ot")
        nc.vector.tensor_tensor(ot[:, :], o2s[:, :], n2w_rep[:, :], op=OP.mult)
        nc.sync.dma_start(out_t[tt * 128:(tt + 1) * 128, :], ot[:, :])



def build_nc(g1, g2, dbg=False):
    from contextlib import ExitStack
    nc = bacc.Bacc("TRN2", target_bir_lowering=False, debug=False,
                   num_devices=NCORES)
    with ExitStack() as ctx:
        tc = ctx.enter_context(tile.TileContext(nc))
        _emit(nc, tc, ctx, g1, g2, dbg)
    nc.compile()
    return nc


def host_prep(inputs):
    bf = ml_dtypes.bfloat16
    x = np.asarray(inputs["x"], np.float32)
    x2d = x.reshape(L, DM)
    w_in = np.asarray(inputs["w_in"], np.float32)
    conv_w = np.asarray(inputs["conv_w"], np.float32)
    conv_b = np.asarray(inputs["conv_b"], np.float32)
    w_xproj = np.asarray(inputs["w_xproj"], np.float32)
    w_dt = np.asarray(inputs["w_dt"], np.float32)
    b_dt = np.asarray(inputs["b_dt"], np.float32)
    A_log = np.asarray(inputs["A_log"], np.float32)
    Dp = np.asarray(inputs["Dp"], np.float32)
    w_out = np.asarray(inputs["w_out"], np.float32)
    n1 = np.asarray(inputs["norm1_w"], np.float32)
    n2 = np.asarray(inputs["norm2_w"], np.float32)
    w1 = np.asarray(inputs["ffn_w1"], np.float32)
    w2 = np.asarray(inputs["ffn_w2"], np.float32)
    b1 = np.asarray(inputs["ffn_b1"], np.float32)
    b2 = np.asarray(inputs["ffn_b2"], np.float32)
    assert np.all(b1 == 0.0) and np.all(b2 == 0.0), "nonzero ffn bias unsupported"

    g1 = float(np.maximum(np.mean(np.abs(w1), dtype=np.float32), 1e-5))
    g2 = float(np.maximum(np.mean(np.abs(w2), dtype=np.float32), 1e-5))
    w1q = np.clip(np.rint(w1 / g1), -1.0, 1.0).astype(np.float32)
    w2q = np.clip(np.rint(w2 / g2), -1.0, 1.0).astype(np.float32)

    xT_bf = np.ascontiguousarray(x2d.T).astype(bf)
    woutT_bf = np.ascontiguousarray(w_out.T).astype(bf)
    w1qT_bf = np.ascontiguousarray(w1q.T).astype(bf)
    w2qT_bf = np.ascontiguousarray(w2q.T).astype(bf)
    n1r = np.ascontiguousarray(n1.reshape(1, DM))
    n2r = np.ascontiguousarray(n2.reshape(1, DM))
    A = -np.exp(A_log)

    in_maps = []
    for c in range(NCORES):
        ch = slice(c * DIC, (c + 1) * DIC)
        w_sel = np.concatenate([w_in[c * DIC:(c + 1) * DIC],
                                w_in[DI + c * DIC:DI + (c + 1) * DIC]], axis=0)
        in_maps.append({
            "xT": xT_bf,
            "x_tok": np.ascontiguousarray(x2d[c * LT:(c + 1) * LT]),
            "winT": np.ascontiguousarray(w_sel.T).astype(bf),
            "convw": np.ascontiguousarray(conv_w[ch, 0, :]),
            "convb": np.ascontiguousarray(conv_b[ch].reshape(-1, 1)),
            "wxpT": np.ascontiguousarray(w_xproj[:, ch].T).astype(bf),
            "wdtT": np.ascontiguousarray(w_dt[ch, :].T).astype(bf),
            "bdt": np.ascontiguousarray(b_dt[ch].reshape(-1, 1)),
            "acol": np.ascontiguousarray(A[ch, :]),
            "dpv": np.ascontiguousarray(Dp[ch].reshape(-1, 1)),
            "woutT": woutT_bf,
            "n1w": n1r,
            "n2w": n2r,
            "w1qT": w1qT_bf,
            "w2qT": w2qT_bf,
        })
    return in_maps, g1, g2


def kernel(**inputs) -> np.ndarray:
    in_maps, g1, g2 = host_prep(inputs)
    key = (round(g1, 10), round(g2, 10))
    if key not in _NC_CACHE:
        _NC_CACHE[key] = build_nc(g1, g2)
    nc = _NC_CACHE[key]
    res = run_bass_kernel_spmd(nc, in_maps, core_ids=list(range(NCORES)))
    out = np.concatenate([res.results[c]["out"] for c in range(NCORES)], axis=0)
    return np.ascontiguousarray(out.reshape(1, L, DM).astype(np.float32))
